# revision 10
# baseline (speedup 1.0000x reference)
"""Distributed causal self-attention kernel for one TRN2 chip (8 NeuronCores).

Self-contained: accepts the FULL inputs of reference.setup_inputs(),
shards internally (tensor-parallel over heads: core c computes heads
(2c, 2c+1) for both batches), runs a Bass/Tile kernel SPMD on cores 0-7
with one 8-core AllToAll to reshard head-split -> token-split before the
output projection, and gathers the full [2, 2048, 1024] output.

v2: fused software pipeline — per (b, qb) block the kernel issues the
Q/K/V chains, then S^T + exp per key-tile pair with the AV accumulation
lagging one group behind, so the PE fills Act-engine exp latency with
matmul work and vice versa. Diagonal key-tiles are causally truncated
(matmul, exp, and AV all skip the masked query columns). Attention
operands are bf16; the proj bias rides the Act engine's bias port.

Compiled graph is cached at module level; first call compiles, later
calls just execute.
"""

import numpy as np
import ml_dtypes
import concourse.bass as bass
import concourse.bacc as bacc
import concourse.tile as tile
import concourse.mybir as mybir

F32 = mybir.dt.float32
BF16 = mybir.dt.bfloat16
Exp = mybir.ActivationFunctionType.Exp
Idn = mybir.ActivationFunctionType.Identity

B, T, C, H, HS = 2, 2048, 1024, 16, 64
NCORES = 8
TLOC = 512         # tokens per core after A2A
NKC = C // 128     # contraction tiles
NJT = T // 128     # key tiles per batch
NQB = T // 512     # query blocks per batch
SCALE = 1.0 / np.sqrt(HS)
PT_BUFS = 12       # exp-output (p) pool depth


def build_nc(timeline=False, repeat=1, phases=("main", "a2a", "proj")):
    nc = bacc.Bacc("TRN2", target_bir_lowering=False, debug=False,
                   num_devices=1 if timeline else NCORES)
    xtb_d = nc.dram_tensor("xtb", [C, B * T], BF16, kind="ExternalInput")
    wqk_d = nc.dram_tensor("wqk", [C, 256], BF16, kind="ExternalInput")
    wvb_d = nc.dram_tensor("wvb", [C, 130], BF16, kind="ExternalInput")
    bvb_d = nc.dram_tensor("bvb", [1, 130], BF16, kind="ExternalInput")
    bqk_d = nc.dram_tensor("bqk", [128, 2], F32, kind="ExternalInput")
    wp_d = nc.dram_tensor("wp", [C, C], BF16, kind="ExternalInput")
    bp_d = nc.dram_tensor("bp", [128, 8], F32, kind="ExternalInput")
    out_d = nc.dram_tensor("out", [C, TLOC], F32, kind="ExternalOutput")

    with tile.TileContext(nc) as tc:
        for _rep in range(repeat):
            _body(nc, tc, xtb_d, wqk_d, wvb_d, bvb_d, bqk_d, wp_d, bp_d,
                  out_d, timeline=timeline, phases=phases)
    nc.compile()
    return nc


def _body(nc, tc, xtb_d, wqk_d, wvb_d, bvb_d, bqk_d, wp_d, bp_d, out_d,
          timeline=False, phases=("main", "a2a", "proj")):
    with (
        tc.tile_pool(name="pers", bufs=1) as pers,
        tc.tile_pool(name="dram", bufs=1, space="DRAM") as dram,
    ):
        a2a_in = dram.tile([C, TLOC], BF16, name="a2a_in")
        a2a_out = dram.tile([C, TLOC], BF16, name="a2a_out")

        wqk = pers.tile([128, NKC, 256], BF16, name="wqk")
        wvb = pers.tile([128, NKC, 130], BF16, name="wvb")
        bvb = pers.tile([1, 130], BF16, name="bvb")
        bqk = pers.tile([128, 2], F32, name="bqk")
        bp = pers.tile([128, 8], F32, name="bp")
        wp = pers.tile([128, NKC, C], BF16, name="wp")
        onesb = pers.tile([1, 128], BF16, name="onesb")
        wrm = pers.tile([1, 1], F32, name="wrm")
        tri = pers.tile([128, 128], BF16, name="tri")
        qt = [[pers.tile([128, 512], BF16, name=f"qt_{b}_{qb}")
               for qb in range(NQB)] for b in range(B)]
        kt = [[pers.tile([128, 512], BF16, name=f"kt_{b}_{qb}")
               for qb in range(NQB)] for b in range(B)]
        va = [[pers.tile([128, 130], BF16, name=f"va_{b}_{jt}")
               for jt in range(NJT)] for b in range(B)]
        ynall = pers.tile([64, 16, TLOC], BF16, name="ynall")

        # weights ride the scalar-engine DMA queue so they never block the
        # sync queue, which is reserved for the large per-rep x loads
        nc.scalar.dma_start(
            out=wqk[:], in_=wqk_d.ap().rearrange("(kc p) m -> p kc m", p=128))
        nc.scalar.dma_start(
            out=wvb[:], in_=wvb_d.ap().rearrange("(kc p) m -> p kc m", p=128))
        nc.scalar.dma_start(out=bvb[:], in_=bvb_d[:])
        nc.scalar.dma_start(out=bqk[:], in_=bqk_d[:])
        nc.scalar.dma_start(out=bp[:], in_=bp_d[:])
        nc.scalar.dma_start(
            out=wp[:], in_=wp_d.ap().rearrange("(kc p) m -> p kc m", p=128))
        nc.vector.memset(onesb[:], 1.0)
        nc.vector.memset(wrm[:], 0.0)
        # warm the exp table set early
        nc.scalar.activation(wrm[:], wrm[:], Exp)
        # tri[j, q] = 1 where j <= q else 0
        nc.gpsimd.memset(tri[:], 0.0)
        nc.gpsimd.affine_select(
            out=tri[:], in_=tri[:],
            compare_op=mybir.AluOpType.is_gt, fill=1.0,
            base=0, pattern=[[-1, 128]], channel_multiplier=1,
        )

        # ---------------- fused QKV + attention ----------------
        if "main" not in phases:
            return
        with (
            tc.tile_pool(name="p2", bufs=1) as p2,
            tc.tile_pool(name="ps2", bufs=1, space="PSUM") as ps2,
        ):
            xts = [p2.tile([128, B * T], BF16, name=f"xts_{kc}")
                   for kc in range(NKC)]
            for kc in range(NKC):
                nc.sync.dma_start(out=xts[kc][:],
                                  in_=xtb_d[128 * kc:128 * (kc + 1), :])

            def issue_qkv(b, qb):
                tok0 = T * b + 512 * qb
                # Q and K chains for this query block
                for m in range(2):
                    qk_ps = ps2.tile([128, 512], F32, tag="qk", bufs=2,
                                     name=f"qkps_{m}_{b}_{qb}")
                    for kc in range(NKC):
                        nc.tensor.matmul(
                            qk_ps[:],
                            wqk[:, kc, 128 * m:128 * (m + 1)],
                            xts[kc][:, tok0:tok0 + 512],
                            start=(kc == 0), stop=(kc == NKC - 1))
                    dst = (qt if m == 0 else kt)[b][qb]
                    nc.scalar.activation(dst[:], qk_ps[:], Idn,
                                         bias=bqk[:, m:m + 1])

                # V tiles tt = 4qb .. 4qb+3 (keys this block unlocks)
                for o in range(4):
                    tt = 4 * qb + o
                    v_ps = ps2.tile([128, 512], F32, tag="qk", bufs=2,
                                    name=f"vps_{b}_{tt}")
                    for kc in range(NKC):
                        nc.tensor.matmul(
                            v_ps[:, 0:130],
                            xts[kc][:, tok0 + 128 * o:tok0 + 128 * (o + 1)],
                            wvb[:, kc, :],
                            start=(kc == 0), stop=False)
                    nc.tensor.matmul(v_ps[:, 0:130], onesb[:], bvb[:],
                                     start=False, stop=True)
                    nc.vector.tensor_copy(va[b][tt][:], v_ps[:, 0:130])

            blocks = [(b, qb) for b in range(B) for qb in range(NQB)]
            issue_qkv(*blocks[0])
            for bi, (b, qb) in enumerate(blocks):
                if bi + 1 < len(blocks):
                    issue_qkv(*blocks[bi + 1])
                if True:
                    njt = 4 * (qb + 1)
                    njg = njt // 2

                    # attention: S^T + exp per jg, AV lagging one jg behind
                    yps = [ps2.tile([65, 512], F32, tag="yt", bufs=2,
                                    name=f"yps_{b}_{qb}_{h}")
                           for h in range(2)]
                    ptl = {}

                    def issue_av(jg, b=b, qb=qb, njt=njt, yps=yps, ptl=ptl):
                        for h in range(2):
                            for jj in range(2):
                                jt = 2 * jg + jj
                                off = 128 * (jt - 4 * qb) if jt >= 4 * qb else 0
                                base = 512 * jj
                                nc.tensor.matmul(
                                    yps[h][:, off:512],
                                    va[b][jt][:, 65 * h:65 * (h + 1)],
                                    ptl[(h, jg)][:, base + off:base + 512],
                                    start=(jt == 0), stop=(jt == njt - 1),
                                    skip_group_check=True)

                    for jg in range(njg):
                        is_diag = jg >= 2 * qb
                        stp = [ps2.tile([128, 1024], F32, tag="st", bufs=2,
                                        name=f"st_{b}_{qb}_{jg}_{h}")
                               for h in range(2)]
                        for jj in range(2):
                            jt = 2 * jg + jj
                            col0 = 128 * (jt - 4 * qb) if jt >= 4 * qb else 0
                            for h in range(2):
                                nc.tensor.matmul(
                                    stp[h][:, 512 * jj + col0:512 * (jj + 1)],
                                    kt[b][jt // 4][
                                        64 * h:64 * (h + 1),
                                        128 * (jt % 4):128 * (jt % 4 + 1)],
                                    qt[b][qb][64 * h:64 * (h + 1), col0:512],
                                    start=True, stop=True,
                                    tile_position=(64 * h, 0))
                        for h in range(2):
                            ptile = p2.tile([128, 1024], BF16, tag="pt",
                                            bufs=PT_BUFS,
                                            name=f"pt_{b}_{qb}_{jg}_{h}")
                            if is_diag:
                                for jj in range(2):
                                    jt = 2 * jg + jj
                                    col0 = 128 * (jt - 4 * qb)
                                    nc.scalar.activation(
                                        ptile[:, 512 * jj + col0:512 * (jj + 1)],
                                        stp[h][:, 512 * jj + col0:512 * (jj + 1)],
                                        Exp, scale=float(SCALE))
                                for jj in range(2):
                                    jt = 2 * jg + jj
                                    col0 = 128 * (jt - 4 * qb)
                                    sl = slice(512 * jj + col0,
                                               512 * jj + col0 + 128)
                                    nc.vector.tensor_mul(ptile[:, sl],
                                                         ptile[:, sl], tri[:])
                            else:
                                nc.scalar.activation(ptile[:], stp[h][:], Exp,
                                                     scale=float(SCALE))
                            ptl[(h, jg)] = ptile
                        if jg >= 1:
                            issue_av(jg - 1)
                    issue_av(njg - 1)

                    # normalize: yn = yps[0:64] / yps[64]
                    for h in range(2):
                        recip = p2.tile([1, 512], F32, tag="recip", bufs=2,
                                        name=f"recip_{b}_{qb}_{h}")
                        rbc = p2.tile([64, 512], F32, tag="rbc", bufs=2,
                                      name=f"rbc_{b}_{qb}_{h}")
                        yn = ynall[:, 2 * (4 * b + qb) + h, :]
                        nc.vector.reciprocal(recip[:], yps[h][64:65, :])
                        nc.gpsimd.partition_broadcast(rbc[:], recip[:])
                        nc.vector.tensor_mul(yn, yps[h][0:64, :], rbc[:])

                    # stream this block's two head-slices into the a2a
                    # staging buffer so only the last block's copy sits on
                    # the collective's critical path
                    tg = 4 * b + qb
                    nc.gpsimd.dma_start(
                        out=a2a_in[128 * tg:128 * (tg + 1), :]
                        .rearrange("(j p) t -> p j t", p=64),
                        in_=ynall[:, 2 * tg:2 * tg + 2, :])

        # ---------------- A2A + c_proj ----------------
        if "a2a" not in phases:
            return
        if timeline:
            # stand-in for the A2A so the single-core cost model runs
            nc.sync.dma_start(out=a2a_out[:], in_=a2a_in[:])
        else:
            nc.gpsimd.collective_compute(
                "AllToAll", mybir.AluOpType.bypass,
                replica_groups=[list(range(NCORES))],
                ins=[a2a_in.opt()], outs=[a2a_out.opt()])

        if "proj" not in phases:
            return
        with (
            tc.tile_pool(name="p3", bufs=1) as p3,
            tc.tile_pool(name="ps3", bufs=1, space="PSUM") as ps3,
        ):
            yls = p3.tile([128, NKC, TLOC], BF16, name="yls")
            nc.scalar.dma_start(
                out=yls[:],
                in_=a2a_out[:].rearrange("(kc p) t -> p kc t", p=128))
            osball = p3.tile([128, 8, TLOC], F32, name="osball")
            out_r = out_d.ap().rearrange("(m p) t -> p m t", p=128)
            for m in range(8):
                pj = ps3.tile([128, TLOC], F32, tag="pj", bufs=4,
                              name=f"pj_{m}")
                for kc in range(NKC):
                    nc.tensor.matmul(
                        pj[:],
                        wp[:, kc, 128 * m:128 * (m + 1)],
                        yls[:, kc, :],
                        start=(kc == 0), stop=(kc == NKC - 1))
                nc.scalar.activation(osball[:, m, :], pj[:], Idn,
                                     bias=bp[:, m:m + 1])
                nc.gpsimd.dma_start(out=out_r[:, m:m + 1, :],
                                    in_=osball[:, m:m + 1, :])


def prep_inputs(x, W_attn, b_attn, W_proj, b_proj):
    """Full inputs -> list of 8 per-core input dicts."""
    x = np.asarray(x, dtype=np.float32)
    W_attn = np.asarray(W_attn, dtype=np.float32)
    b_attn = np.asarray(b_attn, dtype=np.float32)
    W_proj = np.asarray(W_proj, dtype=np.float32)
    b_proj = np.asarray(b_proj, dtype=np.float32)
    bf16 = ml_dtypes.bfloat16
    xtb = np.ascontiguousarray(
        np.concatenate([x[0].T, x[1].T], axis=1).astype(bf16))
    in_maps = []
    for c in range(NCORES):
        h0, h1 = 2 * c, 2 * c + 1
        qcols = np.r_[64 * h0:64 * h0 + 64, 64 * h1:64 * h1 + 64]
        kcols = C + qcols
        vcols = 2 * C + qcols
        wqk = np.concatenate([W_attn[:, qcols], W_attn[:, kcols]], axis=1)
        wvb = np.zeros((C, 130), np.float32)
        wvb[:, 0:64] = W_attn[:, vcols[0:64]]
        wvb[:, 65:129] = W_attn[:, vcols[64:128]]
        bvb = np.zeros((1, 130), np.float32)
        bvb[0, 0:64] = b_attn[vcols[0:64]]
        bvb[0, 65:129] = b_attn[vcols[64:128]]
        bvb[0, 64] = 1.0
        bvb[0, 129] = 1.0
        bqk = np.stack([b_attn[qcols], b_attn[kcols]], axis=1)
        in_maps.append({
            "xtb": xtb,
            "wqk": np.ascontiguousarray(wqk.astype(bf16)),
            "wvb": np.ascontiguousarray(wvb.astype(bf16)),
            "bvb": np.ascontiguousarray(bvb.astype(bf16)),
            "bqk": np.ascontiguousarray(bqk.astype(np.float32)),
            "wp": np.ascontiguousarray(W_proj.astype(bf16)),
            "bp": np.ascontiguousarray(
                b_proj.reshape(8, 128).T.astype(np.float32)),
        })
    return in_maps


def assemble(results):
    """Per-core {'out': [C, TLOC]} -> full [B, T, C]."""
    out = np.empty((B, T, C), dtype=np.float32)
    for c in range(NCORES):
        b, g = c // 4, c % 4
        out[b, TLOC * g:TLOC * (g + 1), :] = results[c]["out"].T
    return out


_CACHE = {}


def kernel(x, W_attn, b_attn, W_proj, b_proj):
    from concourse.bass_utils import run_bass_kernel_spmd

    if "nc" not in _CACHE:
        _CACHE["nc"] = build_nc()
    nc = _CACHE["nc"]
    in_maps = prep_inputs(x, W_attn, b_attn, W_proj, b_proj)
    res = run_bass_kernel_spmd(nc, in_maps, core_ids=list(range(NCORES)))
    return assemble(res.results)


# revision 13
# speedup vs baseline: 1.3044x; 1.3044x over previous
"""Distributed causal self-attention kernel for one TRN2 chip (8 NeuronCores).

Self-contained: accepts the FULL inputs of reference.setup_inputs(),
shards internally (tensor-parallel over heads: core c computes heads
(2c, 2c+1) for both batches), runs a Bass/Tile kernel SPMD on cores 0-7
with one 8-core AllToAll to reshard head-split -> token-split before the
output projection, and gathers the full [2, 2048, 1024] output.

v2: fused software pipeline — per (b, qb) block the kernel issues the
Q/K/V chains, then S^T + exp per key-tile pair with the AV accumulation
lagging one group behind, so the PE fills Act-engine exp latency with
matmul work and vice versa. Diagonal key-tiles are causally truncated
(matmul, exp, and AV all skip the masked query columns). Attention
operands are bf16; the proj bias rides the Act engine's bias port.

Compiled graph is cached at module level; first call compiles, later
calls just execute.
"""

import numpy as np
import ml_dtypes
import concourse.bass as bass
import concourse.bacc as bacc
import concourse.tile as tile
import concourse.mybir as mybir

F32 = mybir.dt.float32
BF16 = mybir.dt.bfloat16
Exp = mybir.ActivationFunctionType.Exp
Idn = mybir.ActivationFunctionType.Identity

B, T, C, H, HS = 2, 2048, 1024, 16, 64
NCORES = 8
TLOC = 512         # tokens per core after A2A
NKC = C // 128     # contraction tiles
NJT = T // 128     # key tiles per batch
NQB = T // 512     # query blocks per batch
SCALE = 1.0 / np.sqrt(HS)
PT_BUFS = 12       # exp-output (p) pool depth


def build_nc(timeline=False, repeat=1, phases=("main", "a2a", "proj")):
    nc = bacc.Bacc("TRN2", target_bir_lowering=False, debug=False,
                   num_devices=1 if timeline else NCORES)
    xtb_d = nc.dram_tensor("xtb", [C, B * T], BF16, kind="ExternalInput")
    wqk_d = nc.dram_tensor("wqk", [C, 256], BF16, kind="ExternalInput")
    wvb_d = nc.dram_tensor("wvb", [C, 130], BF16, kind="ExternalInput")
    bvb_d = nc.dram_tensor("bvb", [1, 130], BF16, kind="ExternalInput")
    bqk_d = nc.dram_tensor("bqk", [128, 2], F32, kind="ExternalInput")
    wp_d = nc.dram_tensor("wp", [C, C], BF16, kind="ExternalInput")
    bp_d = nc.dram_tensor("bp", [128, 8], F32, kind="ExternalInput")
    out_d = nc.dram_tensor("out", [C, TLOC], F32, kind="ExternalOutput")

    with tile.TileContext(nc) as tc:
        for _rep in range(repeat):
            _body(nc, tc, xtb_d, wqk_d, wvb_d, bvb_d, bqk_d, wp_d, bp_d,
                  out_d, timeline=timeline, phases=phases)
    nc.compile()
    return nc


def _body(nc, tc, xtb_d, wqk_d, wvb_d, bvb_d, bqk_d, wp_d, bp_d, out_d,
          timeline=False, phases=("main", "a2a", "proj")):
    with (
        tc.tile_pool(name="pers", bufs=1) as pers,
        tc.tile_pool(name="dram", bufs=1, space="DRAM") as dram,
    ):
        a2a_in = dram.tile([C, TLOC], BF16, name="a2a_in")
        a2a_out = dram.tile([C, TLOC], BF16, name="a2a_out")

        wqk = pers.tile([128, NKC, 256], BF16, name="wqk")
        wvb = pers.tile([128, NKC, 130], BF16, name="wvb")
        bvb = pers.tile([1, 130], BF16, name="bvb")
        bqk = pers.tile([128, 2], F32, name="bqk")
        bp = pers.tile([128, 8], F32, name="bp")
        wp = pers.tile([128, NKC, C], BF16, name="wp")
        onesb = pers.tile([1, 128], BF16, name="onesb")
        wrm = pers.tile([1, 1], F32, name="wrm")
        tri = pers.tile([128, 128], BF16, name="tri")
        qt = [[pers.tile([128, 512], BF16, name=f"qt_{b}_{qb}")
               for qb in range(NQB)] for b in range(B)]
        kt = [[pers.tile([128, 512], BF16, name=f"kt_{b}_{qb}")
               for qb in range(NQB)] for b in range(B)]
        va = [[pers.tile([128, 130], BF16, name=f"va_{b}_{jt}")
               for jt in range(NJT)] for b in range(B)]
        ynall = pers.tile([64, 16, TLOC], BF16, name="ynall")

        # weights ride the scalar-engine DMA queue so they never block the
        # sync queue, which is reserved for the large per-rep x loads
        nc.scalar.dma_start(
            out=wqk[:], in_=wqk_d.ap().rearrange("(kc p) m -> p kc m", p=128))
        nc.scalar.dma_start(
            out=wvb[:], in_=wvb_d.ap().rearrange("(kc p) m -> p kc m", p=128))
        nc.scalar.dma_start(out=bvb[:], in_=bvb_d[:])
        nc.scalar.dma_start(out=bqk[:], in_=bqk_d[:])
        nc.scalar.dma_start(out=bp[:], in_=bp_d[:])
        nc.scalar.dma_start(
            out=wp[:], in_=wp_d.ap().rearrange("(kc p) m -> p kc m", p=128))
        nc.vector.memset(onesb[:], 1.0)
        nc.vector.memset(wrm[:], 0.0)
        # warm the exp table set early
        nc.scalar.activation(wrm[:], wrm[:], Exp)
        # tri[j, q] = 1 where j <= q else 0
        nc.gpsimd.memset(tri[:], 0.0)
        nc.gpsimd.affine_select(
            out=tri[:], in_=tri[:],
            compare_op=mybir.AluOpType.is_gt, fill=1.0,
            base=0, pattern=[[-1, 128]], channel_multiplier=1,
        )

        # ---------------- fused QKV + attention ----------------
        if "main" not in phases:
            return
        with (
            tc.tile_pool(name="p2", bufs=1) as p2,
            tc.tile_pool(name="ps2", bufs=1, space="PSUM") as ps2,
        ):
            xts = [p2.tile([128, B * T], BF16, name=f"xts_{kc}")
                   for kc in range(NKC)]
            for kc in range(NKC):
                nc.sync.dma_start(out=xts[kc][:],
                                  in_=xtb_d[128 * kc:128 * (kc + 1), :])

            def qkv_chains(b, qb):
                """Six PE chain thunks (Q, K, V0..V3) for one query block,
                to be woven between the previous block's attention groups."""
                tok0 = T * b + 512 * qb

                def qk_chain(m):
                    qk_ps = ps2.tile([128, 512], F32, tag="qk", bufs=2,
                                     name=f"qkps_{m}_{b}_{qb}")
                    for kc in range(NKC):
                        nc.tensor.matmul(
                            qk_ps[:],
                            wqk[:, kc, 128 * m:128 * (m + 1)],
                            xts[kc][:, tok0:tok0 + 512],
                            start=(kc == 0), stop=(kc == NKC - 1))
                    dst = (qt if m == 0 else kt)[b][qb]
                    nc.scalar.activation(dst[:], qk_ps[:], Idn,
                                         bias=bqk[:, m:m + 1])

                def v_chain(o):
                    tt = 4 * qb + o
                    v_ps = ps2.tile([128, 512], F32, tag="qk", bufs=2,
                                    name=f"vps_{b}_{tt}")
                    for kc in range(NKC):
                        nc.tensor.matmul(
                            v_ps[:, 0:130],
                            xts[kc][:, tok0 + 128 * o:tok0 + 128 * (o + 1)],
                            wvb[:, kc, :],
                            start=(kc == 0), stop=False)
                    nc.tensor.matmul(v_ps[:, 0:130], onesb[:], bvb[:],
                                     start=False, stop=True)
                    nc.vector.tensor_copy(va[b][tt][:], v_ps[:, 0:130])

                return ([lambda m=m: qk_chain(m) for m in range(2)]
                        + [lambda o=o: v_chain(o) for o in range(4)])

            blocks = [(b, qb) for b in range(B) for qb in range(NQB)]
            for ch in qkv_chains(*blocks[0]):
                ch()
            for bi, (b, qb) in enumerate(blocks):
                chains = (qkv_chains(*blocks[bi + 1])
                          if bi + 1 < len(blocks) else [])
                ci = [0]

                def pop_chain():
                    if ci[0] < len(chains):
                        chains[ci[0]]()
                        ci[0] += 1

                if True:
                    njt = 4 * (qb + 1)
                    njg = njt // 2

                    # attention: S^T + exp per jg, AV lagging one jg behind
                    yps = [ps2.tile([65, 512], F32, tag="yt", bufs=2,
                                    name=f"yps_{b}_{qb}_{h}")
                           for h in range(2)]
                    ptl = {}

                    def issue_av(jg, b=b, qb=qb, njt=njt, yps=yps, ptl=ptl):
                        for h in range(2):
                            for jj in range(2):
                                jt = 2 * jg + jj
                                off = 128 * (jt - 4 * qb) if jt >= 4 * qb else 0
                                base = 512 * jj
                                nc.tensor.matmul(
                                    yps[h][:, off:512],
                                    va[b][jt][:, 65 * h:65 * (h + 1)],
                                    ptl[(h, jg)][:, base + off:base + 512],
                                    start=(jt == 0), stop=(jt == njt - 1),
                                    skip_group_check=True)

                    for jg in range(njg):
                        is_diag = jg >= 2 * qb
                        stp = [ps2.tile([128, 1024], F32, tag="st", bufs=2,
                                        name=f"st_{b}_{qb}_{jg}_{h}")
                               for h in range(2)]
                        for jj in range(2):
                            jt = 2 * jg + jj
                            col0 = 128 * (jt - 4 * qb) if jt >= 4 * qb else 0
                            for h in range(2):
                                nc.tensor.matmul(
                                    stp[h][:, 512 * jj + col0:512 * (jj + 1)],
                                    kt[b][jt // 4][
                                        64 * h:64 * (h + 1),
                                        128 * (jt % 4):128 * (jt % 4 + 1)],
                                    qt[b][qb][64 * h:64 * (h + 1), col0:512],
                                    start=True, stop=True,
                                    tile_position=(64 * h, 0))
                        for h in range(2):
                            ptile = p2.tile([128, 1024], BF16, tag="pt",
                                            bufs=PT_BUFS,
                                            name=f"pt_{b}_{qb}_{jg}_{h}")
                            if is_diag:
                                for jj in range(2):
                                    jt = 2 * jg + jj
                                    col0 = 128 * (jt - 4 * qb)
                                    nc.scalar.activation(
                                        ptile[:, 512 * jj + col0:512 * (jj + 1)],
                                        stp[h][:, 512 * jj + col0:512 * (jj + 1)],
                                        Exp, scale=float(SCALE))
                                for jj in range(2):
                                    jt = 2 * jg + jj
                                    col0 = 128 * (jt - 4 * qb)
                                    sl = slice(512 * jj + col0,
                                               512 * jj + col0 + 128)
                                    nc.vector.tensor_mul(ptile[:, sl],
                                                         ptile[:, sl], tri[:])
                            else:
                                nc.scalar.activation(ptile[:], stp[h][:], Exp,
                                                     scale=float(SCALE))
                            ptl[(h, jg)] = ptile
                        # weave one of the next block's QKV chains between
                        # the S^T issue and the (lag-2) AV so the PE never
                        # waits out the exp latency
                        pop_chain()
                        if jg >= 2:
                            issue_av(jg - 2)
                    if njg >= 2:
                        pop_chain()
                        issue_av(njg - 2)
                    pop_chain()
                    issue_av(njg - 1)
                    while ci[0] < len(chains):
                        chains[ci[0]]()
                        ci[0] += 1

                    # normalize: yn = yps[0:64] / yps[64]
                    for h in range(2):
                        recip = p2.tile([1, 512], F32, tag="recip", bufs=2,
                                        name=f"recip_{b}_{qb}_{h}")
                        rbc = p2.tile([64, 512], F32, tag="rbc", bufs=2,
                                      name=f"rbc_{b}_{qb}_{h}")
                        yn = ynall[:, 2 * (4 * b + qb) + h, :]
                        nc.vector.reciprocal(recip[:], yps[h][64:65, :])
                        nc.gpsimd.partition_broadcast(rbc[:], recip[:])
                        nc.vector.tensor_mul(yn, yps[h][0:64, :], rbc[:])

                    # stream this block's two head-slices into the a2a
                    # staging buffer so only the last block's copy sits on
                    # the collective's critical path
                    tg = 4 * b + qb
                    nc.gpsimd.dma_start(
                        out=a2a_in[128 * tg:128 * (tg + 1), :]
                        .rearrange("(j p) t -> p j t", p=64),
                        in_=ynall[:, 2 * tg:2 * tg + 2, :])

        # ---------------- A2A + c_proj ----------------
        proj_src = a2a_in
        if "a2a" in phases:
            proj_src = a2a_out
            if timeline:
                # stand-in for the A2A so the single-core cost model runs
                nc.sync.dma_start(out=a2a_out[:], in_=a2a_in[:])
            else:
                nc.gpsimd.collective_compute(
                    "AllToAll", mybir.AluOpType.bypass,
                    replica_groups=[list(range(NCORES))],
                    ins=[a2a_in.opt()], outs=[a2a_out.opt()])

        if "proj" not in phases:
            return
        with (
            tc.tile_pool(name="p3", bufs=1) as p3,
            tc.tile_pool(name="ps3", bufs=1, space="PSUM") as ps3,
        ):
            yls = p3.tile([128, NKC, TLOC], BF16, name="yls")
            nc.scalar.dma_start(
                out=yls[:],
                in_=proj_src[:].rearrange("(kc p) t -> p kc t", p=128))
            osball = p3.tile([128, 8, TLOC], F32, name="osball")
            out_r = out_d.ap().rearrange("(m p) t -> p m t", p=128)
            for m in range(8):
                pj = ps3.tile([128, TLOC], F32, tag="pj", bufs=4,
                              name=f"pj_{m}")
                for kc in range(NKC):
                    nc.tensor.matmul(
                        pj[:],
                        wp[:, kc, 128 * m:128 * (m + 1)],
                        yls[:, kc, :],
                        start=(kc == 0), stop=(kc == NKC - 1))
                nc.scalar.activation(osball[:, m, :], pj[:], Idn,
                                     bias=bp[:, m:m + 1])
                nc.gpsimd.dma_start(out=out_r[:, m:m + 1, :],
                                    in_=osball[:, m:m + 1, :])


def prep_inputs(x, W_attn, b_attn, W_proj, b_proj):
    """Full inputs -> list of 8 per-core input dicts."""
    x = np.asarray(x, dtype=np.float32)
    W_attn = np.asarray(W_attn, dtype=np.float32)
    b_attn = np.asarray(b_attn, dtype=np.float32)
    W_proj = np.asarray(W_proj, dtype=np.float32)
    b_proj = np.asarray(b_proj, dtype=np.float32)
    bf16 = ml_dtypes.bfloat16
    xtb = np.ascontiguousarray(
        np.concatenate([x[0].T, x[1].T], axis=1).astype(bf16))
    in_maps = []
    for c in range(NCORES):
        h0, h1 = 2 * c, 2 * c + 1
        qcols = np.r_[64 * h0:64 * h0 + 64, 64 * h1:64 * h1 + 64]
        kcols = C + qcols
        vcols = 2 * C + qcols
        wqk = np.concatenate([W_attn[:, qcols], W_attn[:, kcols]], axis=1)
        wvb = np.zeros((C, 130), np.float32)
        wvb[:, 0:64] = W_attn[:, vcols[0:64]]
        wvb[:, 65:129] = W_attn[:, vcols[64:128]]
        bvb = np.zeros((1, 130), np.float32)
        bvb[0, 0:64] = b_attn[vcols[0:64]]
        bvb[0, 65:129] = b_attn[vcols[64:128]]
        bvb[0, 64] = 1.0
        bvb[0, 129] = 1.0
        bqk = np.stack([b_attn[qcols], b_attn[kcols]], axis=1)
        in_maps.append({
            "xtb": xtb,
            "wqk": np.ascontiguousarray(wqk.astype(bf16)),
            "wvb": np.ascontiguousarray(wvb.astype(bf16)),
            "bvb": np.ascontiguousarray(bvb.astype(bf16)),
            "bqk": np.ascontiguousarray(bqk.astype(np.float32)),
            "wp": np.ascontiguousarray(W_proj.astype(bf16)),
            "bp": np.ascontiguousarray(
                b_proj.reshape(8, 128).T.astype(np.float32)),
        })
    return in_maps


def assemble(results):
    """Per-core {'out': [C, TLOC]} -> full [B, T, C]."""
    out = np.empty((B, T, C), dtype=np.float32)
    for c in range(NCORES):
        b, g = c // 4, c % 4
        out[b, TLOC * g:TLOC * (g + 1), :] = results[c]["out"].T
    return out


_CACHE = {}


def kernel(x, W_attn, b_attn, W_proj, b_proj):
    from concourse.bass_utils import run_bass_kernel_spmd

    if "nc" not in _CACHE:
        _CACHE["nc"] = build_nc()
    nc = _CACHE["nc"]
    in_maps = prep_inputs(x, W_attn, b_attn, W_proj, b_proj)
    res = run_bass_kernel_spmd(nc, in_maps, core_ids=list(range(NCORES)))
    return assemble(res.results)


# revision 17
# speedup vs baseline: 1.5983x; 1.2253x over previous
"""Distributed causal self-attention kernel for one TRN2 chip (8 NeuronCores).

Self-contained: accepts the FULL inputs of reference.setup_inputs(),
shards internally (tensor-parallel over heads: core c computes heads
(2c, 2c+1) for both batches), runs a Bass/Tile kernel SPMD on cores 0-7
with one 8-core AllToAll to reshard head-split -> token-split before the
output projection, and gathers the full [2, 2048, 1024] output.

v2: fused software pipeline — per (b, qb) block the kernel issues the
Q/K/V chains, then S^T + exp per key-tile pair with the AV accumulation
lagging one group behind, so the PE fills Act-engine exp latency with
matmul work and vice versa. Diagonal key-tiles are causally truncated
(matmul, exp, and AV all skip the masked query columns). Attention
operands are bf16; the proj bias rides the Act engine's bias port.

Compiled graph is cached at module level; first call compiles, later
calls just execute.
"""

import numpy as np
import ml_dtypes
import concourse.bass as bass
import concourse.bacc as bacc
import concourse.tile as tile
import concourse.mybir as mybir

F32 = mybir.dt.float32
BF16 = mybir.dt.bfloat16
Exp = mybir.ActivationFunctionType.Exp
Idn = mybir.ActivationFunctionType.Identity

B, T, C, H, HS = 2, 2048, 1024, 16, 64
NCORES = 8
TLOC = 512         # tokens per core after A2A
NKC = C // 128     # contraction tiles
NJT = T // 128     # key tiles per batch
NQB = T // 512     # query blocks per batch
SCALE = 1.0 / np.sqrt(HS)
PT_BUFS = 12       # exp-output (p) pool depth


def build_nc(timeline=False, repeat=1, phases=("main", "a2a", "proj")):
    nc = bacc.Bacc("TRN2", target_bir_lowering=False, debug=False,
                   num_devices=1 if timeline else NCORES)
    xtb_d = nc.dram_tensor("xtb", [C, B * T], BF16, kind="ExternalInput")
    wqk_d = nc.dram_tensor("wqk", [C, 256], BF16, kind="ExternalInput")
    wvb_d = nc.dram_tensor("wvb", [C, 130], BF16, kind="ExternalInput")
    bvb_d = nc.dram_tensor("bvb", [1, 130], BF16, kind="ExternalInput")
    bqk_d = nc.dram_tensor("bqk", [128, 2], F32, kind="ExternalInput")
    wp_d = nc.dram_tensor("wp", [C, C], BF16, kind="ExternalInput")
    bp_d = nc.dram_tensor("bp", [128, 8], F32, kind="ExternalInput")
    out_d = nc.dram_tensor("out", [C, TLOC], F32, kind="ExternalOutput")

    with tile.TileContext(nc) as tc:
        for _rep in range(repeat):
            _body(nc, tc, xtb_d, wqk_d, wvb_d, bvb_d, bqk_d, wp_d, bp_d,
                  out_d, timeline=timeline, phases=phases)
    nc.compile()
    return nc


def _body(nc, tc, xtb_d, wqk_d, wvb_d, bvb_d, bqk_d, wp_d, bp_d, out_d,
          timeline=False, phases=("main", "a2a", "proj")):
    with (
        tc.tile_pool(name="pers", bufs=1) as pers,
        tc.tile_pool(name="dram", bufs=1, space="DRAM") as dram,
    ):
        a2a_in = dram.tile([C, TLOC], BF16, name="a2a_in")
        a2a_out = dram.tile([C, TLOC], BF16, name="a2a_out")

        wqk = pers.tile([128, NKC, 256], BF16, name="wqk")
        wvb = pers.tile([128, NKC, 130], BF16, name="wvb")
        bvb = pers.tile([1, 130], BF16, name="bvb")
        bqk = pers.tile([128, 2], F32, name="bqk")
        bp = pers.tile([128, 8], F32, name="bp")
        wp = pers.tile([128, NKC, C], BF16, name="wp")
        onesb = pers.tile([1, 128], BF16, name="onesb")
        wrm = pers.tile([1, 1], F32, name="wrm")
        tri = pers.tile([128, 128], BF16, name="tri")
        qt = [[pers.tile([128, 512], BF16, name=f"qt_{b}_{qb}")
               for qb in range(NQB)] for b in range(B)]
        kt = [[pers.tile([128, 512], BF16, name=f"kt_{b}_{qb}")
               for qb in range(NQB)] for b in range(B)]
        va = [[pers.tile([128, 130], BF16, name=f"va_{b}_{jt}")
               for jt in range(NJT)] for b in range(B)]
        ynall = pers.tile([64, 16, TLOC], BF16, name="ynall")

        # weights ride the scalar-engine DMA queue so they never block the
        # sync queue, which is reserved for the large per-rep x loads
        nc.scalar.dma_start(
            out=wqk[:], in_=wqk_d.ap().rearrange("(kc p) m -> p kc m", p=128))
        nc.scalar.dma_start(
            out=wvb[:], in_=wvb_d.ap().rearrange("(kc p) m -> p kc m", p=128))
        nc.scalar.dma_start(out=bvb[:], in_=bvb_d[:])
        nc.scalar.dma_start(out=bqk[:], in_=bqk_d[:])
        nc.scalar.dma_start(out=bp[:], in_=bp_d[:])
        nc.scalar.dma_start(
            out=wp[:], in_=wp_d.ap().rearrange("(kc p) m -> p kc m", p=128))
        nc.vector.memset(onesb[:], 1.0)
        nc.vector.memset(wrm[:], 0.0)
        # warm the exp table set early
        nc.scalar.activation(wrm[:], wrm[:], Exp)
        # tri[j, q] = 1 where j <= q else 0
        nc.gpsimd.memset(tri[:], 0.0)
        nc.gpsimd.affine_select(
            out=tri[:], in_=tri[:],
            compare_op=mybir.AluOpType.is_gt, fill=1.0,
            base=0, pattern=[[-1, 128]], channel_multiplier=1,
        )

        # ---------------- fused QKV + attention ----------------
        if "main" not in phases:
            return
        with (
            tc.tile_pool(name="p2", bufs=1) as p2,
            tc.tile_pool(name="ps2", bufs=1, space="PSUM") as ps2,
        ):
            # per-batch x tiles: the next rep's b0 loads can start as soon as
            # this rep's last b0 reader finishes (~half-way through main)
            xts = [[p2.tile([128, T], BF16, name=f"xts_{bb}_{kc}")
                    for kc in range(NKC)] for bb in range(B)]
            for bb in range(B):
                for kc in range(NKC):
                    nc.sync.dma_start(
                        out=xts[bb][kc][:],
                        in_=xtb_d[128 * kc:128 * (kc + 1),
                                  T * bb:T * (bb + 1)])

            def qkv_chains(b, qb):
                """Six PE chain thunks (Q, K, V0..V3) for one query block,
                to be woven between the previous block's attention groups."""
                tok0 = 512 * qb

                def qk_chain(m):
                    qk_ps = ps2.tile([128, 512], F32, tag="qk", bufs=2,
                                     name=f"qkps_{m}_{b}_{qb}")
                    for kc in range(NKC):
                        nc.tensor.matmul(
                            qk_ps[:],
                            wqk[:, kc, 128 * m:128 * (m + 1)],
                            xts[b][kc][:, tok0:tok0 + 512],
                            start=(kc == 0), stop=(kc == NKC - 1))
                    dst = (qt if m == 0 else kt)[b][qb]
                    nc.scalar.activation(dst[:], qk_ps[:], Idn,
                                         bias=bqk[:, m:m + 1])

                def v_chain(o):
                    tt = 4 * qb + o
                    v_ps = ps2.tile([128, 512], F32, tag="qk", bufs=2,
                                    name=f"vps_{b}_{tt}")
                    for kc in range(NKC):
                        nc.tensor.matmul(
                            v_ps[:, 0:130],
                            xts[b][kc][:, tok0 + 128 * o:tok0 + 128 * (o + 1)],
                            wvb[:, kc, :],
                            start=(kc == 0), stop=False)
                    nc.tensor.matmul(v_ps[:, 0:130], onesb[:], bvb[:],
                                     start=False, stop=True)
                    nc.vector.tensor_copy(va[b][tt][:], v_ps[:, 0:130])

                return ([lambda m=m: qk_chain(m) for m in range(2)]
                        + [lambda o=o: v_chain(o) for o in range(4)])

            blocks = [(b, qb) for b in range(B) for qb in range(NQB)]
            for ch in qkv_chains(*blocks[0]):
                ch()
            for bi, (b, qb) in enumerate(blocks):
                chains = (qkv_chains(*blocks[bi + 1])
                          if bi + 1 < len(blocks) else [])
                ci = [0]

                def pop_chain():
                    if ci[0] < len(chains):
                        chains[ci[0]]()
                        ci[0] += 1

                if True:
                    njt = 4 * (qb + 1)
                    njg = njt // 2

                    # attention: S^T + exp per jg, AV lagging one jg behind
                    yps = [ps2.tile([65, 512], F32, tag="yt", bufs=2,
                                    name=f"yps_{b}_{qb}_{h}")
                           for h in range(2)]
                    ptl = {}

                    def issue_av(jg, b=b, qb=qb, njt=njt, yps=yps, ptl=ptl):
                        for h in range(2):
                            for jj in range(2):
                                jt = 2 * jg + jj
                                off = 128 * (jt - 4 * qb) if jt >= 4 * qb else 0
                                base = 512 * jj
                                nc.tensor.matmul(
                                    yps[h][:, off:512],
                                    va[b][jt][:, 65 * h:65 * (h + 1)],
                                    ptl[(h, jg)][:, base + off:base + 512],
                                    start=(jt == 0), stop=(jt == njt - 1),
                                    skip_group_check=True)

                    for jg in range(njg):
                        is_diag = jg >= 2 * qb
                        stp = [ps2.tile([128, 1024], F32, tag="st", bufs=2,
                                        name=f"st_{b}_{qb}_{jg}_{h}")
                               for h in range(2)]
                        for jj in range(2):
                            jt = 2 * jg + jj
                            col0 = 128 * (jt - 4 * qb) if jt >= 4 * qb else 0
                            for h in range(2):
                                nc.tensor.matmul(
                                    stp[h][:, 512 * jj + col0:512 * (jj + 1)],
                                    kt[b][jt // 4][
                                        64 * h:64 * (h + 1),
                                        128 * (jt % 4):128 * (jt % 4 + 1)],
                                    qt[b][qb][64 * h:64 * (h + 1), col0:512],
                                    start=True, stop=True,
                                    tile_position=(64 * h, 0))
                        for h in range(2):
                            ptile = p2.tile([128, 1024], BF16, tag="pt",
                                            bufs=PT_BUFS,
                                            name=f"pt_{b}_{qb}_{jg}_{h}")
                            if is_diag:
                                for jj in range(2):
                                    jt = 2 * jg + jj
                                    col0 = 128 * (jt - 4 * qb)
                                    nc.scalar.activation(
                                        ptile[:, 512 * jj + col0:512 * (jj + 1)],
                                        stp[h][:, 512 * jj + col0:512 * (jj + 1)],
                                        Exp, scale=float(SCALE))
                                for jj in range(2):
                                    jt = 2 * jg + jj
                                    col0 = 128 * (jt - 4 * qb)
                                    sl = slice(512 * jj + col0,
                                               512 * jj + col0 + 128)
                                    nc.vector.tensor_mul(ptile[:, sl],
                                                         ptile[:, sl], tri[:])
                            else:
                                nc.scalar.activation(ptile[:], stp[h][:], Exp,
                                                     scale=float(SCALE))
                            ptl[(h, jg)] = ptile
                        # weave one of the next block's QKV chains between
                        # the S^T issue and the (lag-2) AV so the PE never
                        # waits out the exp latency
                        pop_chain()
                        if jg >= 2:
                            issue_av(jg - 2)
                    if njg >= 2:
                        pop_chain()
                        issue_av(njg - 2)
                    pop_chain()
                    issue_av(njg - 1)
                    while ci[0] < len(chains):
                        chains[ci[0]]()
                        ci[0] += 1

                    # normalize: yn = yps[0:64] / yps[64]
                    for h in range(2):
                        recip = p2.tile([1, 512], F32, tag="recip", bufs=2,
                                        name=f"recip_{b}_{qb}_{h}")
                        rbc = p2.tile([64, 512], F32, tag="rbc", bufs=2,
                                      name=f"rbc_{b}_{qb}_{h}")
                        yn = ynall[:, 2 * (4 * b + qb) + h, :]
                        nc.vector.reciprocal(recip[:], yps[h][64:65, :])
                        nc.gpsimd.partition_broadcast(rbc[:], recip[:])
                        nc.vector.tensor_mul(yn, yps[h][0:64, :], rbc[:])

                    # stream this block's two head-slices into the a2a
                    # staging buffer so only the last block's copy sits on
                    # the collective's critical path
                    tg = 4 * b + qb
                    nc.gpsimd.dma_start(
                        out=a2a_in[128 * tg:128 * (tg + 1), :]
                        .rearrange("(j p) t -> p j t", p=64),
                        in_=ynall[:, 2 * tg:2 * tg + 2, :])

        # ---------------- A2A + c_proj ----------------
        proj_src = a2a_in
        if "a2a" in phases:
            proj_src = a2a_out
            if timeline:
                # stand-in for the A2A so the single-core cost model runs
                nc.sync.dma_start(out=a2a_out[:], in_=a2a_in[:])
            else:
                nc.gpsimd.collective_compute(
                    "AllToAll", mybir.AluOpType.bypass,
                    replica_groups=[list(range(NCORES))],
                    ins=[a2a_in.opt()], outs=[a2a_out.opt()])

        if "proj" not in phases:
            return
        with (
            tc.tile_pool(name="p3", bufs=1) as p3,
            tc.tile_pool(name="ps3", bufs=1, space="PSUM") as ps3,
        ):
            yls = p3.tile([128, NKC, TLOC], BF16, name="yls")
            src_r = proj_src[:].rearrange("(kc p) t -> p kc t", p=128)
            for kc in range(NKC):
                nc.scalar.dma_start(out=yls[:, kc, :], in_=src_r[:, kc, :])
            osball = p3.tile([128, 8, TLOC], F32, name="osball")
            out_r = out_d.ap().rearrange("(m p) t -> p m t", p=128)
            # kc-outer: all 8 m-accumulators live in PSUM at once, so the
            # first matmuls start as soon as the first 128-row slice lands
            pjs = [ps3.tile([128, TLOC], F32, tag="pj", bufs=8,
                            name=f"pj_{m}") for m in range(8)]
            for kc in range(NKC):
                for m in range(8):
                    nc.tensor.matmul(
                        pjs[m][:],
                        wp[:, kc, 128 * m:128 * (m + 1)],
                        yls[:, kc, :],
                        start=(kc == 0), stop=(kc == NKC - 1),
                        skip_group_check=True)
            for m in range(8):
                nc.scalar.activation(osball[:, m, :], pjs[m][:], Idn,
                                     bias=bp[:, m:m + 1])
                nc.gpsimd.dma_start(out=out_r[:, m:m + 1, :],
                                    in_=osball[:, m:m + 1, :])


def prep_inputs(x, W_attn, b_attn, W_proj, b_proj):
    """Full inputs -> list of 8 per-core input dicts."""
    x = np.asarray(x, dtype=np.float32)
    W_attn = np.asarray(W_attn, dtype=np.float32)
    b_attn = np.asarray(b_attn, dtype=np.float32)
    W_proj = np.asarray(W_proj, dtype=np.float32)
    b_proj = np.asarray(b_proj, dtype=np.float32)
    bf16 = ml_dtypes.bfloat16
    xtb = np.ascontiguousarray(
        np.concatenate([x[0].T, x[1].T], axis=1).astype(bf16))
    in_maps = []
    for c in range(NCORES):
        h0, h1 = 2 * c, 2 * c + 1
        qcols = np.r_[64 * h0:64 * h0 + 64, 64 * h1:64 * h1 + 64]
        kcols = C + qcols
        vcols = 2 * C + qcols
        wqk = np.concatenate([W_attn[:, qcols], W_attn[:, kcols]], axis=1)
        wvb = np.zeros((C, 130), np.float32)
        wvb[:, 0:64] = W_attn[:, vcols[0:64]]
        wvb[:, 65:129] = W_attn[:, vcols[64:128]]
        bvb = np.zeros((1, 130), np.float32)
        bvb[0, 0:64] = b_attn[vcols[0:64]]
        bvb[0, 65:129] = b_attn[vcols[64:128]]
        bvb[0, 64] = 1.0
        bvb[0, 129] = 1.0
        bqk = np.stack([b_attn[qcols], b_attn[kcols]], axis=1)
        in_maps.append({
            "xtb": xtb,
            "wqk": np.ascontiguousarray(wqk.astype(bf16)),
            "wvb": np.ascontiguousarray(wvb.astype(bf16)),
            "bvb": np.ascontiguousarray(bvb.astype(bf16)),
            "bqk": np.ascontiguousarray(bqk.astype(np.float32)),
            "wp": np.ascontiguousarray(W_proj.astype(bf16)),
            "bp": np.ascontiguousarray(
                b_proj.reshape(8, 128).T.astype(np.float32)),
        })
    return in_maps


def assemble(results):
    """Per-core {'out': [C, TLOC]} -> full [B, T, C]."""
    out = np.empty((B, T, C), dtype=np.float32)
    for c in range(NCORES):
        b, g = c // 4, c % 4
        out[b, TLOC * g:TLOC * (g + 1), :] = results[c]["out"].T
    return out


_CACHE = {}


def kernel(x, W_attn, b_attn, W_proj, b_proj):
    from concourse.bass_utils import run_bass_kernel_spmd

    if "nc" not in _CACHE:
        _CACHE["nc"] = build_nc()
    nc = _CACHE["nc"]
    in_maps = prep_inputs(x, W_attn, b_attn, W_proj, b_proj)
    res = run_bass_kernel_spmd(nc, in_maps, core_ids=list(range(NCORES)))
    return assemble(res.results)
